# revision 1
# baseline (speedup 1.0000x reference)
"""Trainium2 Bass kernel for hetero GNN (2x SAGEConv layers + in/out proj).

Full inputs in, full output out. Internally: dst-node sharding across 8
NeuronCores, edge bucketing by (dst block of 128, src quadrant) on host,
device-side gather via SWDGE dma_gather, segment-mean via one-hot matmul
accumulated in PSUM, AllGather collectives for the shared node tables.
"""

import math

import numpy as np

import concourse.bacc as bacc
import concourse.bass as bass
import concourse.mybir as mybir
from concourse import tile
from concourse.bass_utils import run_bass_kernel_spmd

FP32 = mybir.dt.float32
BF16 = mybir.dt.bfloat16
I16 = mybir.dt.int16
AF = mybir.ActivationFunctionType
ALU = mybir.AluOpType

BF16_NP = mybir.dt.np(BF16)


def full_cfg():
    return dict(
        N=100000,
        E=1600000,
        DA=300,
        DU=64,
        H=64,
        OUT=2,
        n_cores=8,
        shard=12544,  # 98 * 128 per-core dst shard
        cq_min=5,
    )


# ----------------------------------------------------------------------------
# Host-side edge preprocessing
# ----------------------------------------------------------------------------


def prep_edges(src, dst, cfg):
    """Bucket edges by (dst block of 128, src quadrant); build gather index /
    one-hot slot / reciprocal-degree arrays per core.

    Returns (CQ, per_core list of dicts with idx_w/slot_w/rval_w).
    """
    N, shard, n_cores = cfg["N"], cfg["shard"], cfg["n_cores"]
    NPAD = n_cores * shard
    QN = NPAD // 4
    assert QN < 32768, QN
    NBLK = shard // 128

    src = np.asarray(src, dtype=np.int64)
    dst = np.asarray(dst, dtype=np.int64)
    deg = np.bincount(dst, minlength=N).astype(np.float64)
    recip = (1.0 / np.maximum(deg, 1.0)).astype(np.float32)

    blk = dst >> 7  # global 128-block id
    quad = src // QN
    n_cells = n_cores * NBLK * 4
    cell = blk * 4 + quad
    # sort edges by cell (order within a cell is irrelevant)
    order = np.argsort(cell, kind="stable")
    c_src = src[order]
    c_dst = dst[order]
    c_cell = cell[order]
    starts = np.searchsorted(c_cell, np.arange(n_cells))
    cnts = np.bincount(c_cell, minlength=n_cells)
    CQ = max(cfg["cq_min"], int(math.ceil(cnts.max() / 128)))
    CB = 4 * CQ

    j = np.arange(len(c_src)) - starts[c_cell]  # position within cell
    loc_idx = (c_src - quad[order] * QN).astype(np.int16)
    slot_val = (c_dst & 127).astype(np.float32)
    rval_val = recip[c_dst]

    b_local_all = (c_cell // 4) % NBLK
    q_all = c_cell % 4
    core_all = c_cell // (4 * NBLK)

    per_core = []
    for c in range(n_cores):
        m = core_all == c
        bl = b_local_all[m]
        q = q_all[m]
        jj = j[m]
        # gather idx array, 16-partition wrapped, replicated 8x
        idx_w = np.zeros((128, NBLK * 4 * CQ * 8), dtype=np.int16)
        col = (bl * 4 + q) * (CQ * 8) + jj // 16
        row = jj % 16
        for g in range(8):
            idx_w[row + 16 * g, col] = loc_idx[m]
        # slot / recip-val arrays: [128, NBLK*CB]
        slot_w = np.full((128, NBLK * CB), 999.0, dtype=np.float32)
        rval_w = np.zeros((128, NBLK * CB), dtype=np.float32)
        colS = bl * CB + q * CQ + jj // 128
        rowS = jj % 128
        slot_w[rowS, colS] = slot_val[m]
        rval_w[rowS, colS] = rval_val[m]
        per_core.append(dict(idx_w=idx_w, slot_w=slot_w, rval_w=rval_w))
    return CQ, per_core


def _lin_bf16(w):
    """[out,in] fp32 -> lhsT layout [in,out] bf16."""
    return np.ascontiguousarray(w.T).astype(BF16_NP)


def _bias_col(b):
    return np.asarray(b, np.float32).reshape(-1, 1)


# ----------------------------------------------------------------------------
# Device program
# ----------------------------------------------------------------------------


def build_program(cfg, CQp, CQb, reps=1, skip=()):
    N, DA, DU, H, OUT = cfg["N"], cfg["DA"], cfg["DU"], cfg["H"], cfg["OUT"]
    n_cores, shard = cfg["n_cores"], cfg["shard"]
    NPAD = n_cores * shard
    QN = NPAD // 4
    NBLK = shard // 128
    CBp, CBb = 4 * CQp, 4 * CQb
    DA_PAD = ((DA + 15) // 16) * 16  # 304
    KA = [(k, min(128, DA_PAD - k)) for k in range(0, DA_PAD, 128)]
    TW = 512  # in-proj / head tile width
    n_tw = [(t, min(TW, shard - t)) for t in range(0, shard, TW)]

    nc = bacc.Bacc("TRN2", debug=False)

    # ---- I/O ----
    xaT = nc.dram_tensor("xaT", [DA_PAD, shard], BF16, kind="ExternalInput")
    xuT = nc.dram_tensor("xuT", [DU, shard], BF16, kind="ExternalInput")
    w_in_aT = nc.dram_tensor("w_in_aT", [DA_PAD, H], BF16, kind="ExternalInput")
    b_in_a = nc.dram_tensor("b_in_a", [H, 1], FP32, kind="ExternalInput")
    w_in_uT = nc.dram_tensor("w_in_uT", [DU, H], BF16, kind="ExternalInput")
    b_in_u = nc.dram_tensor("b_in_u", [H, 1], FP32, kind="ExternalInput")
    convw = {}
    for et in ("c1p", "c1b", "c2p"):
        convw[et] = (
            nc.dram_tensor(f"{et}_wlT", [H, H], BF16, kind="ExternalInput"),
            nc.dram_tensor(f"{et}_bl", [H, 1], FP32, kind="ExternalInput"),
            nc.dram_tensor(f"{et}_wrT", [H, H], BF16, kind="ExternalInput"),
        )
    w_outT = nc.dram_tensor("w_outT", [H, OUT], BF16, kind="ExternalInput")
    b_out = nc.dram_tensor("b_out", [OUT, 1], FP32, kind="ExternalInput")
    iota_in = nc.dram_tensor("iota", [128, 128], FP32, kind="ExternalInput")
    ident_in = nc.dram_tensor("ident", [128, 128], BF16, kind="ExternalInput")
    idx_p = nc.dram_tensor("idx_p", [128, NBLK * 4 * CQp * 8], I16, kind="ExternalInput")
    slot_p = nc.dram_tensor("slot_p", [128, NBLK * CBp], FP32, kind="ExternalInput")
    rval_p = nc.dram_tensor("rval_p", [128, NBLK * CBp], FP32, kind="ExternalInput")
    idx_b = nc.dram_tensor("idx_b", [128, NBLK * 4 * CQb * 8], I16, kind="ExternalInput")
    slot_b = nc.dram_tensor("slot_b", [128, NBLK * CBb], FP32, kind="ExternalInput")
    rval_b = nc.dram_tensor("rval_b", [128, NBLK * CBb], FP32, kind="ExternalInput")
    out_d = nc.dram_tensor("out", [OUT, shard], FP32, kind="ExternalOutput")

    # internal HBM
    u_shard = nc.dram_tensor("u_shard", [shard, 128], BF16)
    a_shard = nc.dram_tensor("a_shard", [shard, 128], BF16)
    u1_shard = nc.dram_tensor("u1_shard", [shard, 128], BF16)
    u_rm = nc.dram_tensor("u_rm", [NPAD, 128], BF16, addr_space="Shared")
    a_rm = nc.dram_tensor("a_rm", [NPAD, 128], BF16, addr_space="Shared")
    u1_rm = nc.dram_tensor("u1_rm", [NPAD, 128], BF16, addr_space="Shared")
    groups = [list(range(n_cores))]

    from contextlib import ExitStack

    with tile.TileContext(nc) as tc, ExitStack() as _stack:
        cpool = _stack.enter_context(tc.tile_pool(name="const", bufs=1))
        # resident constants
        iota_sb = cpool.tile([128, 128], FP32, tag="iota")
        ident_sb = cpool.tile([128, 128], BF16, tag="ident")
        nc.sync.dma_start(iota_sb[:], iota_in[:])
        nc.sync.dma_start(ident_sb[:], ident_in[:])

        def load_const(t, shape, dtype, tag):
            s = cpool.tile(shape, dtype, tag=tag)
            nc.sync.dma_start(s[:], t[:])
            return s

        w_in_aT_s = cpool.tile([128, len(KA), H], BF16, tag="w_in_aT")
        for ki, (k0, kn) in enumerate(KA):
            nc.sync.dma_start(w_in_aT_s[0:kn, ki, :], w_in_aT[k0 : k0 + kn, :])
        b_in_a_s = load_const(b_in_a, [H, 1], FP32, "b_in_a")
        w_in_uT_s = load_const(w_in_uT, [DU, H], BF16, "w_in_uT")
        b_in_u_s = load_const(b_in_u, [H, 1], FP32, "b_in_u")
        convw_s = {}
        for et in ("c1p", "c1b", "c2p"):
            wlT, bl, wrT = convw[et]
            convw_s[et] = (
                load_const(wlT, [H, H], BF16, f"{et}_wlT"),
                load_const(bl, [H, 1], FP32, f"{et}_bl"),
                load_const(wrT, [H, H], BF16, f"{et}_wrT"),
            )
        w_outT_s = load_const(w_outT, [H, OUT], BF16, "w_outT")
        b_out_s = load_const(b_out, [OUT, 1], FP32, "b_out")
        idx_p_s = load_const(idx_p, [128, NBLK * 4 * CQp * 8], I16, "idx_p")
        slot_p_s = load_const(slot_p, [128, NBLK * CBp], FP32, "slot_p")
        rval_p_s = load_const(rval_p, [128, NBLK * CBp], FP32, "rval_p")
        slot_b_s = load_const(slot_b, [128, NBLK * CBb], FP32, "slot_b")
        rval_b_s = load_const(rval_b, [128, NBLK * CBb], FP32, "rval_b")

        # resident feature-major node tables (own shard)
        uT_own = cpool.tile([H, shard], BF16, tag="uT_own")
        aT_own = cpool.tile([H, shard], BF16, tag="aT_own")
        a1T = cpool.tile([H, shard], BF16, tag="a1T")

        def transpose_out(pool_ps, pool_st, src_ap, b, shard_dram):
            """[64,128] feature-major block -> [128,64] -> shard_dram rows."""
            tp = pool_ps.tile([128, H], BF16, tag="tpps")
            nc.tensor.transpose(tp[:], src_ap, ident_sb[0:H, 0:H])
            st = pool_st.tile([128, H], BF16, tag="tpst")
            nc.scalar.copy(st[:], tp[:])
            nc.sync.dma_start(shard_dram[b * 128 : (b + 1) * 128, 0:H], st[:])

        # ------------------- stage 1: input projections -------------------
        def _inproj():
          with (
            tc.tile_pool(name="ip_ps", bufs=3, space="PSUM") as ip_ps,
            tc.tile_pool(name="tp_ps", bufs=2, space="PSUM") as tp_ps,
            tc.tile_pool(name="ip_sb", bufs=4) as ip_sb,
            tc.tile_pool(name="tp_sb", bufs=3) as tp_sb,
        ):
            for t0, tw in n_tw:
                xt = ip_sb.tile([DU, TW], BF16, tag="xu")
                nc.sync.dma_start(xt[:, 0:tw], xuT[:, t0 : t0 + tw])
                ps = ip_ps.tile([H, TW], FP32, tag="ipps")
                nc.tensor.matmul(ps[:, 0:tw], w_in_uT_s[:], xt[:, 0:tw])
                nc.scalar.activation(
                    uT_own[:, t0 : t0 + tw], ps[:, 0:tw], AF.Relu, bias=b_in_u_s[:]
                )
            for t0, tw in n_tw:
                ps = ip_ps.tile([H, TW], FP32, tag="ipps")
                for ki, (k0, kn) in enumerate(KA):
                    xt = ip_sb.tile([128, TW], BF16, tag="xa")
                    nc.sync.dma_start(xt[0:kn, 0:tw], xaT[k0 : k0 + kn, t0 : t0 + tw])
                    nc.tensor.matmul(
                        ps[:, 0:tw],
                        w_in_aT_s[0:kn, ki, :],
                        xt[0:kn, 0:tw],
                        start=(ki == 0),
                        stop=(ki == len(KA) - 1),
                    )
                nc.scalar.activation(
                    aT_own[:, t0 : t0 + tw], ps[:, 0:tw], AF.Relu, bias=b_in_a_s[:]
                )
            for b in range(NBLK):
                transpose_out(tp_ps, tp_sb, uT_own[:, b * 128 : (b + 1) * 128], b, u_shard)
                transpose_out(tp_ps, tp_sb, aT_own[:, b * 128 : (b + 1) * 128], b, a_shard)

        # ------------------- all-gather u, a -------------------
        def _ag_ua():
            if "ag" in skip:
                return
            nc.gpsimd.collective_compute(
                "AllGather", ALU.bypass, replica_groups=groups,
                ins=[u_shard[:]], outs=[u_rm[:]],
            )
            nc.gpsimd.collective_compute(
                "AllGather", ALU.bypass, replica_groups=groups,
                ins=[a_shard[:]], outs=[a_rm[:]],
            )

        # ------------------- conv layers -------------------
        def conv_layer(
            pools, gtable, idx_res, idx_dram, slot_s, rval_s, CQ, et, xdstT,
            outT, relu, shard_dram, head,
        ):
            CB = 4 * CQ
            (msg_p, s_p, agg_ps, lin_ps, agg_sb, ctp_ps, ctp_sb, outb_p,
             idx_pool, hd_ps, hd_sb) = pools
            wlT_s, bl_s, wrT_s = convw_s[et]
            for b in range(NBLK):
                if idx_res is not None:
                    idxt = idx_res[:, b * 4 * CQ * 8 : (b + 1) * 4 * CQ * 8]
                else:
                    it = idx_pool.tile([128, 4 * CQ * 8], I16, tag="idxs")
                    nc.sync.dma_start(
                        it[:], idx_dram[:, b * 4 * CQ * 8 : (b + 1) * 4 * CQ * 8]
                    )
                    idxt = it[:]
                msg = msg_p.tile([128, CB, 128], BF16, tag="msg")
                if "gather" not in skip:
                    for q in range(4):
                        nc.gpsimd.dma_gather(
                            msg[:, q * CQ : (q + 1) * CQ, :],
                            gtable[q * QN : (q + 1) * QN, :],
                            idxt[:, q * CQ * 8 : (q + 1) * CQ * 8],
                            CQ * 128,
                            CQ * 128,
                            128,
                        )
                elif b == 0:
                    nc.vector.memset(msg[:], 0.0)
                agg = agg_ps.tile([H, 128], FP32, tag="agg")
                for c in range(CB):
                    S = s_p.tile([128, 128], BF16, tag="S")
                    nc.vector.tensor_scalar(
                        S[:],
                        iota_sb[:],
                        slot_s[:, b * CB + c : b * CB + c + 1],
                        rval_s[:, b * CB + c : b * CB + c + 1],
                        ALU.is_equal,
                        ALU.mult,
                    )
                    nc.tensor.matmul(
                        agg[:],
                        msg[:, c, 0:H],
                        S[:],
                        start=(c == 0),
                        stop=(c == CB - 1),
                    )
                aggs = agg_sb.tile([H, 128], BF16, tag="aggs")
                nc.scalar.copy(aggs[:], agg[:])
                lin = lin_ps.tile([H, 128], FP32, tag="lin")
                nc.tensor.matmul(lin[:], wlT_s[:], aggs[:], start=True, stop=False)
                nc.tensor.matmul(
                    lin[:],
                    wrT_s[:],
                    xdstT[:, b * 128 : (b + 1) * 128],
                    start=False,
                    stop=True,
                )
                if outT is not None:
                    ovec = outT[:, b * 128 : (b + 1) * 128]
                else:
                    ob = outb_p.tile([H, 128], BF16, tag="outb")
                    ovec = ob[:]
                if relu:
                    nc.scalar.activation(ovec, lin[:], AF.Relu, bias=bl_s[:])
                else:
                    nc.vector.tensor_scalar_add(ovec, lin[:], bl_s[:])
                if shard_dram is not None:
                    transpose_out(ctp_ps, ctp_sb, ovec, b, shard_dram)
                if head:
                    hp = hd_ps.tile([OUT, 128], FP32, tag="hdps")
                    nc.tensor.matmul(hp[:], w_outT_s[:], ovec)
                    ho = hd_sb.tile([OUT, 128], FP32, tag="hdo")
                    nc.vector.tensor_scalar_add(ho[:], hp[:], b_out_s[:])
                    nc.sync.dma_start(out_d[:, b * 128 : (b + 1) * 128], ho[:])

        def _convs():
          with (
            tc.tile_pool(name="msg", bufs=3) as msg_p,
            tc.tile_pool(name="S", bufs=4) as s_p,
            tc.tile_pool(name="agg_ps", bufs=2, space="PSUM") as agg_ps,
            tc.tile_pool(name="lin_ps", bufs=2, space="PSUM") as lin_ps,
            tc.tile_pool(name="agg_sb", bufs=3) as agg_sb,
            tc.tile_pool(name="ctp_ps", bufs=2, space="PSUM") as ctp_ps,
            tc.tile_pool(name="ctp_sb", bufs=3) as ctp_sb,
            tc.tile_pool(name="outb", bufs=3) as outb_p,
            tc.tile_pool(name="idxs", bufs=3) as idx_pool,
            tc.tile_pool(name="hd_ps", bufs=2, space="PSUM") as hd_ps,
            tc.tile_pool(name="hd_sb", bufs=3) as hd_sb,
        ):
            pools = (msg_p, s_p, agg_ps, lin_ps, agg_sb, ctp_ps, ctp_sb,
                     outb_p, idx_pool, hd_ps, hd_sb)
            # users first so the u1 all-gather overlaps the articles conv
            conv_layer(
                pools, a_rm, None, idx_b, slot_b_s, rval_b_s, CQb, "c1b",
                uT_own, None, True, u1_shard, False,
            )
            if "ag" not in skip:
                nc.gpsimd.collective_compute(
                    "AllGather", ALU.bypass, replica_groups=groups,
                    ins=[u1_shard[:]], outs=[u1_rm[:]],
                )
            conv_layer(
                pools, u_rm, idx_p_s, None, slot_p_s, rval_p_s, CQp, "c1p",
                aT_own, a1T, True, None, False,
            )
            conv_layer(
                pools, u1_rm, idx_p_s, None, slot_p_s, rval_p_s, CQp, "c2p",
                a1T, None, False, None, True,
            )

        for _rep in range(reps):
            _inproj()
            _ag_ua()
            if "convs" not in skip:
                _convs()

    nc.compile()
    return nc


# ----------------------------------------------------------------------------
# Entry point
# ----------------------------------------------------------------------------

_CACHE = {}


def build_in_maps(inputs, cfg, CQp, per_core_p, CQb, per_core_b):
    N, DA, DU, H = cfg["N"], cfg["DA"], cfg["DU"], cfg["H"]
    n_cores, shard = cfg["n_cores"], cfg["shard"]
    DA_PAD = ((DA + 15) // 16) * 16
    xa = np.asarray(inputs["x_article"], np.float32)
    xu = np.asarray(inputs["x_user"], np.float32)

    shared = dict(
        w_in_aT=np.concatenate(
            [_lin_bf16(inputs["w_in_a"]), np.zeros((DA_PAD - DA, H), BF16_NP)], 0
        ),
        b_in_a=_bias_col(inputs["b_in_a"]),
        w_in_uT=_lin_bf16(inputs["w_in_u"]),
        b_in_u=_bias_col(inputs["b_in_u"]),
        w_outT=_lin_bf16(inputs["w_out"]),
        b_out=_bias_col(inputs["b_out"]),
        iota=np.tile(np.arange(128, dtype=np.float32), (128, 1)),
        ident=np.eye(128, dtype=BF16_NP),
    )
    for et, pfx in (("c1p", "c1p"), ("c1b", "c1b"), ("c2p", "c2p")):
        shared[f"{et}_wlT"] = _lin_bf16(inputs[f"{pfx}_wl"])
        shared[f"{et}_bl"] = _bias_col(inputs[f"{pfx}_bl"])
        shared[f"{et}_wrT"] = _lin_bf16(inputs[f"{pfx}_wr"])

    in_maps = []
    for c in range(n_cores):
        c0, c1 = c * shard, min((c + 1) * shard, N)
        xaT_c = np.zeros((DA_PAD, shard), BF16_NP)
        xaT_c[:DA, : c1 - c0] = xa[c0:c1].T.astype(BF16_NP)
        xuT_c = np.zeros((DU, shard), BF16_NP)
        xuT_c[:, : c1 - c0] = xu[c0:c1].T.astype(BF16_NP)
        m = dict(shared)
        m["xaT"] = xaT_c
        m["xuT"] = xuT_c
        m["idx_p"] = per_core_p[c]["idx_w"]
        m["slot_p"] = per_core_p[c]["slot_w"]
        m["rval_p"] = per_core_p[c]["rval_w"]
        m["idx_b"] = per_core_b[c]["idx_w"]
        m["slot_b"] = per_core_b[c]["slot_w"]
        m["rval_b"] = per_core_b[c]["rval_w"]
        in_maps.append(m)
    return in_maps


def _run(inputs, cfg, trace=False, reps=1):
    N, n_cores, shard = cfg["N"], cfg["n_cores"], cfg["shard"]

    CQp, per_core_p = prep_edges(inputs["ei_posts"][0], inputs["ei_posts"][1], cfg)
    CQb, per_core_b = prep_edges(inputs["ei_pb"][0], inputs["ei_pb"][1], cfg)

    key = (tuple(sorted(cfg.items())), CQp, CQb, reps)
    if key not in _CACHE:
        _CACHE[key] = build_program(cfg, CQp, CQb, reps)
    nc = _CACHE[key]

    in_maps = build_in_maps(inputs, cfg, CQp, per_core_p, CQb, per_core_b)

    res = run_bass_kernel_spmd(nc, in_maps, list(range(n_cores)), trace=trace)
    outs = [res.results[c]["out"] for c in range(n_cores)]  # [2, shard] each
    full = np.concatenate(outs, axis=1)[:, :N].T.astype(np.float32)
    return np.ascontiguousarray(full), res


def kernel(**inputs):
    out, _ = _run(inputs, full_cfg(), trace=False)
    return out



# revision 11
# speedup vs baseline: 1.2993x; 1.2993x over previous
"""Trainium2 Bass kernel for hetero GNN (2x SAGEConv layers + in/out proj).

Full inputs in, full output out. Internally: dst-node sharding across 8
NeuronCores, edge bucketing by (dst block of 128, src quadrant) on host,
device-side gather via SWDGE dma_gather (batched over block groups),
segment-mean via one-hot matmul accumulated in PSUM, AllGather collectives
for the shared node tables.

v2 structure: the ei_posts edge list is gathered ONCE from a combined
[u | u1] node table (256B rows, fully used), so conv1-posts and conv2-posts
share gather descriptors, one-hot S tiles, and accumulation matmuls (the
c2p aggregation rides in partitions 64:128 of the same PSUM tile). Only two
AllGathers (a, u_comb) and two gather passes (ei_pb, ei_posts) remain.
Input projections emit both feature-major (for lin_r) and node-major (for
the gather tables) layouts directly via per-block matmuls with a ones-row
bias trick, eliminating on-chip transposes for u and a.
"""

import math

import numpy as np

import concourse.bacc as bacc
import concourse.bass as bass
import concourse.mybir as mybir
from concourse import tile
from concourse.bass_utils import run_bass_kernel_spmd

FP32 = mybir.dt.float32
BF16 = mybir.dt.bfloat16
I16 = mybir.dt.int16
AF = mybir.ActivationFunctionType
ALU = mybir.AluOpType

BF16_NP = mybir.dt.np(BF16)


def full_cfg():
    return dict(
        N=100000,
        E=1600000,
        DA=300,
        DU=64,
        H=64,
        OUT=2,
        n_cores=8,
        shard=12544,  # 98 * 128 per-core dst shard
        cq_min=5,
    )


import os


def _pick_G(nblk):
    if os.environ.get("KERNEL_G"):
        return min(int(os.environ["KERNEL_G"]), nblk)
    if nblk % 7 == 0:
        return 7
    return min(8, nblk)


# ----------------------------------------------------------------------------
# Host-side edge preprocessing
# ----------------------------------------------------------------------------


def prep_edges(src, dst, cfg):
    """Bucket edges by (dst block of 128, src quadrant); build gather index /
    one-hot slot / reciprocal-degree arrays per core.

    idx layout groups gather indices by (block-group, quadrant) so one
    dma_gather covers G blocks of one quadrant.

    Returns (CQ, per_core list of dicts with idx_w/slot_w/rval_w).
    """
    N, shard, n_cores = cfg["N"], cfg["shard"], cfg["n_cores"]
    NPAD = n_cores * shard
    QN = NPAD // 4
    assert QN < 32768, QN
    NBLK = shard // 128
    G = _pick_G(NBLK)
    n_groups = (NBLK + G - 1) // G

    src = np.asarray(src, dtype=np.int64)
    dst = np.asarray(dst, dtype=np.int64)
    deg = np.bincount(dst, minlength=N).astype(np.float64)
    recip = (1.0 / np.maximum(deg, 1.0)).astype(np.float32)

    blk = dst >> 7  # global 128-block id
    quad = src // QN
    cell = blk * 4 + quad
    n_cells = n_cores * NBLK * 4
    order = np.argsort(cell, kind="stable")
    c_src = src[order]
    c_dst = dst[order]
    c_cell = cell[order]
    starts = np.searchsorted(c_cell, np.arange(n_cells))
    cnts = np.bincount(c_cell, minlength=n_cells)
    CQ = max(cfg["cq_min"], int(math.ceil(cnts.max() / 128)))
    CB = 4 * CQ

    j = np.arange(len(c_src)) - starts[c_cell]  # position within cell
    loc_idx = (c_src - quad[order] * QN).astype(np.int16)
    slot_val = (c_dst & 127).astype(np.float32)
    rval_val = recip[c_dst]

    b_local_all = (c_cell // 4) % NBLK
    q_all = c_cell % 4
    core_all = c_cell // (4 * NBLK)

    per_core = []
    for c in range(n_cores):
        m = core_all == c
        bl = b_local_all[m]
        q = q_all[m]
        jj = j[m]
        g = bl // G
        bl_in_g = bl % G
        # gather idx array, 16-partition wrapped, replicated 8x;
        # grouped so (group, quadrant) segments are contiguous.
        idx_w = np.zeros((128, n_groups * 4 * G * CQ * 8), dtype=np.int16)
        col = ((g * 4 + q) * G + bl_in_g) * (CQ * 8) + jj // 16
        row = jj % 16
        for r in range(8):
            idx_w[row + 16 * r, col] = loc_idx[m]
        # slot / recip-val arrays: [128, NBLK*CB] bf16
        slot_w = np.full((128, NBLK * CB), 999.0, dtype=np.float32)
        rval_w = np.zeros((128, NBLK * CB), dtype=np.float32)
        colS = bl * CB + q * CQ + jj // 128
        rowS = jj % 128
        slot_w[rowS, colS] = slot_val[m]
        rval_w[rowS, colS] = rval_val[m]
        per_core.append(dict(idx_w=idx_w, slot_w=slot_w, rval_w=rval_w))
    return CQ, per_core


def _lin_bf16(w):
    """[out,in] fp32 -> lhsT layout [in,out] bf16."""
    return np.ascontiguousarray(np.asarray(w).T).astype(BF16_NP)


def _lin_bias_bf16(w, b):
    """[out,in] fp32 + [out] bias -> [in+1, out] bf16 with bias row."""
    w = np.asarray(w, np.float32)
    b = np.asarray(b, np.float32)
    return np.concatenate([w.T, b.reshape(1, -1)], axis=0).astype(BF16_NP)


def _bias_col(b):
    return np.asarray(b, np.float32).reshape(-1, 1)


# ----------------------------------------------------------------------------
# Device program
# ----------------------------------------------------------------------------


def build_program(cfg, CQp, CQb, reps=1, skip=()):
    N, DA, DU, H, OUT = cfg["N"], cfg["DA"], cfg["DU"], cfg["H"], cfg["OUT"]
    n_cores, shard = cfg["n_cores"], cfg["shard"]
    NPAD = n_cores * shard
    QN = NPAD // 4
    NBLK = shard // 128
    G = _pick_G(NBLK)
    n_groups = (NBLK + G - 1) // G
    CBp, CBb = 4 * CQp, 4 * CQb
    DU1 = DU + 1  # ones row for bias
    DA1 = DA + 1
    KA = [(k, min(128, DA1 - k)) for k in range(0, DA1, 128)]
    TW = 512  # feature-major in-proj tile width
    n_tw = [(t, min(TW, shard - t)) for t in range(0, shard, TW)]

    nc = bacc.Bacc("TRN2", debug=False)

    # ---- I/O ----
    xaT = nc.dram_tensor("xaT", [DA1, shard], BF16, kind="ExternalInput")
    xuT = nc.dram_tensor("xuT", [DU1, shard], BF16, kind="ExternalInput")
    w_in_a = nc.dram_tensor("w_in_a", [DA1, H], BF16, kind="ExternalInput")
    w_in_u = nc.dram_tensor("w_in_u", [DU1, H], BF16, kind="ExternalInput")
    convw = {}
    for et in ("c1p", "c1b", "c2p"):
        convw[et] = (
            nc.dram_tensor(f"{et}_wlT", [H, H], BF16, kind="ExternalInput"),
            nc.dram_tensor(f"{et}_bl", [H, 1], FP32, kind="ExternalInput"),
            nc.dram_tensor(f"{et}_wrT", [H, H], BF16, kind="ExternalInput"),
        )
    # c2p wl staged in partitions 64:128 so its lhsT base matches aggs[64:128]
    c2p_wlT_hi = nc.dram_tensor("c2p_wlT_hi", [128, H], BF16, kind="ExternalInput")
    w_outT = nc.dram_tensor("w_outT", [H, OUT], BF16, kind="ExternalInput")
    b_out = nc.dram_tensor("b_out", [OUT, 1], FP32, kind="ExternalInput")
    iota_in = nc.dram_tensor("iota", [128, 128], BF16, kind="ExternalInput")
    ident_in = nc.dram_tensor("ident", [128, 128], BF16, kind="ExternalInput")
    NIDXp = n_groups * 4 * G * CQp * 8
    NIDXb = n_groups * 4 * G * CQb * 8
    idx_p = nc.dram_tensor("idx_p", [128, NIDXp], I16, kind="ExternalInput")
    slot_p = nc.dram_tensor("slot_p", [128, NBLK * CBp], FP32, kind="ExternalInput")
    rval_p = nc.dram_tensor("rval_p", [128, NBLK * CBp], FP32, kind="ExternalInput")
    idx_b = nc.dram_tensor("idx_b", [128, NIDXb], I16, kind="ExternalInput")
    slot_b = nc.dram_tensor("slot_b", [128, NBLK * CBb], FP32, kind="ExternalInput")
    rval_b = nc.dram_tensor("rval_b", [128, NBLK * CBb], FP32, kind="ExternalInput")
    out_d = nc.dram_tensor("out", [OUT, shard], FP32, kind="ExternalOutput")

    # internal HBM
    a_shard = nc.dram_tensor("a_shard", [shard, 128], BF16)
    uc_shard = nc.dram_tensor("uc_shard", [shard, 128], BF16)
    a_rm = nc.dram_tensor("a_rm", [NPAD, 128], BF16, addr_space="Shared")
    uc_rm = nc.dram_tensor("uc_rm", [NPAD, 128], BF16, addr_space="Shared")
    groups = [list(range(n_cores))]

    from contextlib import ExitStack

    with tile.TileContext(nc) as tc, ExitStack() as _stack:
        cpool = _stack.enter_context(tc.tile_pool(name="const", bufs=1))
        iota_sb = cpool.tile([128, 128], BF16, tag="iota")
        ident_sb = cpool.tile([128, 128], BF16, tag="ident")
        nc.sync.dma_start(iota_sb[:], iota_in[:])
        nc.sync.dma_start(ident_sb[:], ident_in[:])

        def load_const(t, shape, dtype, tag):
            s = cpool.tile(shape, dtype, tag=tag)
            nc.sync.dma_start(s[:], t[:])
            return s

        w_in_a_s = cpool.tile([128, len(KA), H], BF16, tag="w_in_a")
        for ki, (k0, kn) in enumerate(KA):
            nc.sync.dma_start(w_in_a_s[0:kn, ki, :], w_in_a[k0 : k0 + kn, :])
        w_in_u_s = load_const(w_in_u, [DU1, H], BF16, "w_in_u")
        convw_s = {}
        for et in ("c1p", "c1b", "c2p"):
            wlT, bl, wrT = convw[et]
            convw_s[et] = (
                load_const(wlT, [H, H], BF16, f"{et}_wlT"),
                load_const(bl, [H, 1], FP32, f"{et}_bl"),
                load_const(wrT, [H, H], BF16, f"{et}_wrT"),
            )
        c2p_wlT_hi_s = load_const(c2p_wlT_hi, [128, H], BF16, "c2p_wlT_hi")
        w_outT_s = load_const(w_outT, [H, OUT], BF16, "w_outT")
        b_out_s = load_const(b_out, [OUT, 1], FP32, "b_out")
        slot_p_s = load_const(slot_p, [128, NBLK * CBp], FP32, "slot_p")
        rval_p_s = load_const(rval_p, [128, NBLK * CBp], FP32, "rval_p")
        slot_b_s = load_const(slot_b, [128, NBLK * CBb], FP32, "slot_b")
        rval_b_s = load_const(rval_b, [128, NBLK * CBb], FP32, "rval_b")

        # resident feature-major node tables (own shard)
        uT_own = cpool.tile([H, shard], BF16, tag="uT_own")
        aT_own = cpool.tile([H, shard], BF16, tag="aT_own")

        # ------------------- stage 1: input projections -------------------
        def _inproj():
          with (
            tc.tile_pool(name="ip_ps", bufs=3, space="PSUM") as ip_ps,
            tc.tile_pool(name="nm_ps", bufs=3, space="PSUM") as nm_ps,
            tc.tile_pool(name="ip_sb", bufs=3) as ip_sb,
            tc.tile_pool(name="nm_sb", bufs=4) as nm_sb,
        ):
            # articles first: their node-major rows feed the a AllGather,
            # which gates pass 1.
            for t0, tw in n_tw:
                xt = ip_sb.tile([128, len(KA), TW], BF16, tag="xa")
                for ki, (k0, kn) in enumerate(KA):
                    nc.sync.dma_start(
                        xt[0:kn, ki, 0:tw], xaT[k0 : k0 + kn, t0 : t0 + tw]
                    )
                # feature-major: aT_own[:, tile] = relu(W' @ xa')
                ps = ip_ps.tile([H, TW], FP32, tag="ipps")
                for ki, (k0, kn) in enumerate(KA):
                    nc.tensor.matmul(
                        ps[:, 0:tw],
                        w_in_a_s[0:kn, ki, :],
                        xt[0:kn, ki, 0:tw],
                        start=(ki == 0),
                        stop=(ki == len(KA) - 1),
                    )
                nc.scalar.activation(aT_own[:, t0 : t0 + tw], ps[:, 0:tw], AF.Relu)
                # node-major: a_shard rows = relu(xa'^T @ W')
                for b0 in range(0, tw, 128):
                    ps2 = nm_ps.tile([128, H], FP32, tag="nmps")
                    for ki, (k0, kn) in enumerate(KA):
                        nc.tensor.matmul(
                            ps2[:],
                            xt[0:kn, ki, b0 : b0 + 128],
                            w_in_a_s[0:kn, ki, :],
                            start=(ki == 0),
                            stop=(ki == len(KA) - 1),
                        )
                    st = nm_sb.tile([128, H], BF16, tag="nmst")
                    nc.scalar.activation(st[:], ps2[:], AF.Relu)
                    nc.sync.dma_start(
                        a_shard[t0 + b0 : t0 + b0 + 128, 0:H], st[:]
                    )
            if "ag" not in skip:
                nc.gpsimd.collective_compute(
                    "AllGather", ALU.bypass, replica_groups=groups,
                    ins=[a_shard[:]], outs=[a_rm[:]],
                )
            for t0, tw in n_tw:
                xt = ip_sb.tile([DU1, TW], BF16, tag="xu")
                nc.sync.dma_start(xt[:, 0:tw], xuT[:, t0 : t0 + tw])
                ps = ip_ps.tile([H, TW], FP32, tag="ipps")
                nc.tensor.matmul(ps[:, 0:tw], w_in_u_s[:], xt[:, 0:tw])
                nc.scalar.activation(uT_own[:, t0 : t0 + tw], ps[:, 0:tw], AF.Relu)
                for b0 in range(0, tw, 128):
                    ps2 = nm_ps.tile([128, H], FP32, tag="nmps")
                    nc.tensor.matmul(ps2[:], xt[:, b0 : b0 + 128], w_in_u_s[:])
                    st = nm_sb.tile([128, H], BF16, tag="nmst")
                    nc.scalar.activation(st[:], ps2[:], AF.Relu)
                    nc.sync.dma_start(
                        uc_shard[t0 + b0 : t0 + b0 + 128, 0:H], st[:]
                    )

        # ------------------- conv passes -------------------
        def conv_pass(
            pools, gtable, idx_dram, slot_s, rval_s, CQ, pass2,
        ):
            """pass2=False: c1b (agg a over ei_pb -> u1 rows into uc_shard).
            pass2=True: c1p + c2p + head (agg [u|u1] over ei_posts)."""
            CB = 4 * CQ
            (msg_p, s_p, agg_ps, lin_ps, agg_sb, tp_ps, tp_sb,
             idx_pool, hd_ps, ost_p) = pools
            FW = 128 if pass2 else H
            for g in range(n_groups):
                g0 = g * G
                Gg = min(G, NBLK - g0)
                it = idx_pool.tile([128, 4 * G * CQ * 8], I16, tag="idxs")
                nc.sync.dma_start(
                    it[:, 0 : 4 * G * CQ * 8],
                    idx_dram[:, g * 4 * G * CQ * 8 : (g + 1) * 4 * G * CQ * 8],
                )
                msg = msg_p.tile([128, 4 * G * CQ, 128], BF16, tag="msg")
                if "gather" not in skip:
                    for q in range(4):
                        nc.gpsimd.dma_gather(
                            msg[:, q * G * CQ : q * G * CQ + Gg * CQ, :],
                            gtable[q * QN : (q + 1) * QN, :],
                            it[:, q * G * CQ * 8 : q * G * CQ * 8 + Gg * CQ * 8],
                            Gg * CQ * 128,
                            Gg * CQ * 128,
                            128,
                            # single_packet concatenation hangs the SDMA for
                            # per-engine packets past ~4KB; plain packets work.
                            single_packet=False,
                        )
                else:
                    nc.vector.memset(msg[:], 0.0)
                if pass2:
                    ost = ost_p.tile([OUT, G * 128], FP32, tag="ost")
                for bl in range(Gg):
                    b = g0 + bl
                    agg = agg_ps.tile([FW, 128], FP32, tag="agg")
                    for c in range(CB):
                        q, cj = divmod(c, CQ)
                        S = s_p.tile([128, 128], BF16, tag="S")
                        nc.vector.tensor_scalar(
                            S[:],
                            iota_sb[:],
                            slot_s[:, b * CB + c : b * CB + c + 1],
                            rval_s[:, b * CB + c : b * CB + c + 1],
                            ALU.is_equal,
                            ALU.mult,
                        )
                        nc.tensor.matmul(
                            agg[:],
                            msg[:, q * G * CQ + bl * CQ + cj, 0:FW],
                            S[:],
                            start=(c == 0),
                            stop=(c == CB - 1),
                        )
                    aggs = agg_sb.tile([FW, 128], BF16, tag="aggs")
                    nc.scalar.copy(aggs[:], agg[:])
                    if not pass2:
                        wlT_s, bl_s, wrT_s = convw_s["c1b"]
                        lin = lin_ps.tile([H, 128], FP32, tag="lin")
                        nc.tensor.matmul(lin[:], wlT_s[:], aggs[:],
                                         start=True, stop=False)
                        nc.tensor.matmul(
                            lin[:], wrT_s[:], uT_own[:, b * 128 : (b + 1) * 128],
                            start=False, stop=True,
                        )
                        u1 = agg_sb.tile([H, 128], BF16, tag="u1")
                        nc.scalar.activation(u1[:], lin[:], AF.Relu, bias=bl_s[:])
                        tp = tp_ps.tile([128, H], BF16, tag="tpps")
                        nc.tensor.transpose(tp[:], u1[:], ident_sb[0:H, 0:H])
                        st = tp_sb.tile([128, H], BF16, tag="tpst")
                        nc.scalar.copy(st[:], tp[:])
                        nc.sync.dma_start(
                            uc_shard[b * 128 : (b + 1) * 128, H:128], st[:]
                        )
                    else:
                        wlT_s, bl_s, wrT_s = convw_s["c1p"]
                        lin = lin_ps.tile([H, 128], FP32, tag="lin")
                        nc.tensor.matmul(lin[:], wlT_s[:], aggs[0:H, :],
                                         start=True, stop=False)
                        nc.tensor.matmul(
                            lin[:], wrT_s[:], aT_own[:, b * 128 : (b + 1) * 128],
                            start=False, stop=True,
                        )
                        a1 = agg_sb.tile([H, 128], BF16, tag="a1")
                        nc.scalar.activation(a1[:], lin[:], AF.Relu, bias=bl_s[:])
                        _, bl2, wrT2 = convw_s["c2p"]
                        lin2 = lin_ps.tile([H, 128], FP32, tag="lin")
                        nc.tensor.matmul(lin2[:], c2p_wlT_hi_s[H:128, :],
                                         aggs[H:128, :], start=True, stop=False)
                        nc.tensor.matmul(lin2[:], wrT2[:], a1[:],
                                         start=False, stop=True)
                        a2 = agg_sb.tile([H, 128], BF16, tag="a2")
                        nc.vector.tensor_scalar_add(a2[:], lin2[:], bl2[:])
                        hp = hd_ps.tile([OUT, 128], FP32, tag="hdps")
                        nc.tensor.matmul(hp[:], w_outT_s[:], a2[:])
                        nc.vector.tensor_scalar_add(
                            ost[:, bl * 128 : (bl + 1) * 128], hp[:], b_out_s[:]
                        )
                if pass2:
                    nc.sync.dma_start(
                        out_d[:, g0 * 128 : g0 * 128 + Gg * 128],
                        ost[:, 0 : Gg * 128],
                    )

        def _convs():
          with (
            tc.tile_pool(name="msg", bufs=2) as msg_p,
            tc.tile_pool(name="S", bufs=6) as s_p,
            tc.tile_pool(name="agg_ps", bufs=2, space="PSUM") as agg_ps,
            tc.tile_pool(name="lin_ps", bufs=2, space="PSUM") as lin_ps,
            tc.tile_pool(name="agg_sb", bufs=3) as agg_sb,
            tc.tile_pool(name="tp_ps", bufs=2, space="PSUM") as tp_ps,
            tc.tile_pool(name="tp_sb", bufs=3) as tp_sb,
            tc.tile_pool(name="idxs", bufs=2) as idx_pool,
            tc.tile_pool(name="hd_ps", bufs=2, space="PSUM") as hd_ps,
            tc.tile_pool(name="ost", bufs=2) as ost_p,
        ):
            pools = (msg_p, s_p, agg_ps, lin_ps, agg_sb, tp_ps, tp_sb,
                     idx_pool, hd_ps, ost_p)
            # pass 1: c1b over ei_pb -> u1 rows into uc_shard[:, 64:128]
            conv_pass(pools, a_rm, idx_b, slot_b_s, rval_b_s, CQb, False)
            if "ag" not in skip:
                nc.gpsimd.collective_compute(
                    "AllGather", ALU.bypass, replica_groups=groups,
                    ins=[uc_shard[:]], outs=[uc_rm[:]],
                )
            # pass 2: c1p + c2p + head over ei_posts
            conv_pass(pools, uc_rm, idx_p, slot_p_s, rval_p_s, CQp, True)

        for _rep in range(reps):
            _inproj()
            if "convs" not in skip:
                _convs()

    nc.compile()
    return nc


# ----------------------------------------------------------------------------
# Entry point
# ----------------------------------------------------------------------------

_CACHE = {}


def build_in_maps(inputs, cfg, CQp, per_core_p, CQb, per_core_b):
    N, DA, DU, H = cfg["N"], cfg["DA"], cfg["DU"], cfg["H"]
    n_cores, shard = cfg["n_cores"], cfg["shard"]
    DA1, DU1 = DA + 1, DU + 1
    xa = np.asarray(inputs["x_article"], np.float32)
    xu = np.asarray(inputs["x_user"], np.float32)

    shared = dict(
        w_in_a=_lin_bias_bf16(inputs["w_in_a"], inputs["b_in_a"]),
        w_in_u=_lin_bias_bf16(inputs["w_in_u"], inputs["b_in_u"]),
        w_outT=_lin_bf16(inputs["w_out"]),
        b_out=_bias_col(inputs["b_out"]),
        iota=np.tile(np.arange(128, dtype=np.float32), (128, 1)).astype(BF16_NP),
        ident=np.eye(128, dtype=BF16_NP),
    )
    for et in ("c1p", "c1b", "c2p"):
        shared[f"{et}_wlT"] = _lin_bf16(inputs[f"{et}_wl"])
        shared[f"{et}_bl"] = _bias_col(inputs[f"{et}_bl"])
        shared[f"{et}_wrT"] = _lin_bf16(inputs[f"{et}_wr"])
    shared["c2p_wlT_hi"] = np.concatenate(
        [np.zeros((H, H), BF16_NP), _lin_bf16(inputs["c2p_wl"])], axis=0
    )

    in_maps = []
    for c in range(n_cores):
        c0, c1 = c * shard, min((c + 1) * shard, N)
        xaT_c = np.zeros((DA1, shard), BF16_NP)
        xaT_c[:DA, : c1 - c0] = xa[c0:c1].T.astype(BF16_NP)
        xaT_c[DA, :] = 1.0
        xuT_c = np.zeros((DU1, shard), BF16_NP)
        xuT_c[:DU, : c1 - c0] = xu[c0:c1].T.astype(BF16_NP)
        xuT_c[DU, :] = 1.0
        m = dict(shared)
        m["xaT"] = xaT_c
        m["xuT"] = xuT_c
        m["idx_p"] = per_core_p[c]["idx_w"]
        m["slot_p"] = per_core_p[c]["slot_w"]
        m["rval_p"] = per_core_p[c]["rval_w"]
        m["idx_b"] = per_core_b[c]["idx_w"]
        m["slot_b"] = per_core_b[c]["slot_w"]
        m["rval_b"] = per_core_b[c]["rval_w"]
        in_maps.append(m)
    return in_maps


def _run(inputs, cfg, trace=False, reps=1):
    N, n_cores, shard = cfg["N"], cfg["n_cores"], cfg["shard"]

    CQp, per_core_p = prep_edges(inputs["ei_posts"][0], inputs["ei_posts"][1], cfg)
    CQb, per_core_b = prep_edges(inputs["ei_pb"][0], inputs["ei_pb"][1], cfg)

    key = (tuple(sorted(cfg.items())), CQp, CQb, reps)
    if key not in _CACHE:
        _CACHE[key] = build_program(cfg, CQp, CQb, reps)
    nc = _CACHE[key]

    in_maps = build_in_maps(inputs, cfg, CQp, per_core_p, CQb, per_core_b)

    res = run_bass_kernel_spmd(nc, in_maps, list(range(n_cores)), trace=trace)
    outs = [res.results[c]["out"] for c in range(n_cores)]  # [2, shard] each
    full = np.concatenate(outs, axis=1)[:, :N].T.astype(np.float32)
    return np.ascontiguousarray(full), res


def kernel(**inputs):
    out, _ = _run(inputs, full_cfg(), trace=False)
    return out


# revision 14
# speedup vs baseline: 1.3194x; 1.0154x over previous
"""Trainium2 Bass kernel for hetero GNN (2x SAGEConv layers + in/out proj).

Full inputs in, full output out. Internally: dst-node sharding across 8
NeuronCores, edge bucketing by (dst block of 128, src quadrant) on host,
device-side gather via SWDGE dma_gather (batched over block groups),
segment-mean via one-hot matmul accumulated in PSUM, AllGather collectives
for the shared node tables.

v2 structure: the ei_posts edge list is gathered ONCE from a combined
[u | u1] node table (256B rows, fully used), so conv1-posts and conv2-posts
share gather descriptors, one-hot S tiles, and accumulation matmuls (the
c2p aggregation rides in partitions 64:128 of the same PSUM tile). Only two
AllGathers (a, u_comb) and two gather passes (ei_pb, ei_posts) remain.
Input projections emit both feature-major (for lin_r) and node-major (for
the gather tables) layouts directly via per-block matmuls with a ones-row
bias trick, eliminating on-chip transposes for u and a.
"""

import math

import numpy as np

import concourse.bacc as bacc
import concourse.bass as bass
import concourse.mybir as mybir
from concourse import tile
from concourse.bass_utils import run_bass_kernel_spmd

FP32 = mybir.dt.float32
BF16 = mybir.dt.bfloat16
I16 = mybir.dt.int16
AF = mybir.ActivationFunctionType
ALU = mybir.AluOpType

BF16_NP = mybir.dt.np(BF16)


def full_cfg():
    return dict(
        N=100000,
        E=1600000,
        DA=300,
        DU=64,
        H=64,
        OUT=2,
        n_cores=8,
        shard=12544,  # 98 * 128 per-core dst shard
        cq_min=5,
    )


import os


def _pick_G(nblk):
    """Blocks per dma_gather call. G=1 (one 128-dst block, CQ*128 indices per
    call) measures fastest on HW: it keeps single_packet concatenation safe,
    which halves the per-descriptor HBM latency cost; the extra SWDGE
    descriptor-generation calls hide entirely under the gather DMA time."""
    if os.environ.get("KERNEL_G"):
        return min(int(os.environ["KERNEL_G"]), nblk)
    return 1


# ----------------------------------------------------------------------------
# Host-side edge preprocessing
# ----------------------------------------------------------------------------


def prep_edges(src, dst, cfg):
    """Bucket edges by (dst block of 128, src quadrant); build gather index /
    one-hot slot / reciprocal-degree arrays per core.

    idx layout groups gather indices by (block-group, quadrant) so one
    dma_gather covers G blocks of one quadrant.

    Returns (CQ, per_core list of dicts with idx_w/slot_w/rval_w).
    """
    N, shard, n_cores = cfg["N"], cfg["shard"], cfg["n_cores"]
    NPAD = n_cores * shard
    QN = NPAD // 4
    assert QN < 32768, QN
    NBLK = shard // 128
    G = _pick_G(NBLK)
    n_groups = (NBLK + G - 1) // G

    src = np.asarray(src, dtype=np.int64)
    dst = np.asarray(dst, dtype=np.int64)
    deg = np.bincount(dst, minlength=N).astype(np.float64)
    recip = (1.0 / np.maximum(deg, 1.0)).astype(np.float32)

    blk = dst >> 7  # global 128-block id
    quad = src // QN
    cell = blk * 4 + quad
    n_cells = n_cores * NBLK * 4
    order = np.argsort(cell, kind="stable")
    c_src = src[order]
    c_dst = dst[order]
    c_cell = cell[order]
    starts = np.searchsorted(c_cell, np.arange(n_cells))
    cnts = np.bincount(c_cell, minlength=n_cells)
    CQ = max(cfg["cq_min"], int(math.ceil(cnts.max() / 128)))
    CB = 4 * CQ

    j = np.arange(len(c_src)) - starts[c_cell]  # position within cell
    loc_idx = (c_src - quad[order] * QN).astype(np.int16)
    slot_val = (c_dst & 127).astype(np.float32)
    rval_val = recip[c_dst]

    b_local_all = (c_cell // 4) % NBLK
    q_all = c_cell % 4
    core_all = c_cell // (4 * NBLK)

    per_core = []
    for c in range(n_cores):
        m = core_all == c
        bl = b_local_all[m]
        q = q_all[m]
        jj = j[m]
        g = bl // G
        bl_in_g = bl % G
        # gather idx array, 16-partition wrapped, replicated 8x;
        # grouped so (group, quadrant) segments are contiguous.
        idx_w = np.zeros((128, n_groups * 4 * G * CQ * 8), dtype=np.int16)
        col = ((g * 4 + q) * G + bl_in_g) * (CQ * 8) + jj // 16
        row = jj % 16
        for r in range(8):
            idx_w[row + 16 * r, col] = loc_idx[m]
        # slot / recip-val arrays: [128, NBLK*CB] bf16
        slot_w = np.full((128, NBLK * CB), 999.0, dtype=np.float32)
        rval_w = np.zeros((128, NBLK * CB), dtype=np.float32)
        colS = bl * CB + q * CQ + jj // 128
        rowS = jj % 128
        slot_w[rowS, colS] = slot_val[m]
        rval_w[rowS, colS] = rval_val[m]
        per_core.append(dict(idx_w=idx_w, slot_w=slot_w, rval_w=rval_w))
    return CQ, per_core


def _lin_bf16(w):
    """[out,in] fp32 -> lhsT layout [in,out] bf16."""
    return np.ascontiguousarray(np.asarray(w).T).astype(BF16_NP)


def _lin_bias_bf16(w, b):
    """[out,in] fp32 + [out] bias -> [in+1, out] bf16 with bias row."""
    w = np.asarray(w, np.float32)
    b = np.asarray(b, np.float32)
    return np.concatenate([w.T, b.reshape(1, -1)], axis=0).astype(BF16_NP)


def _bias_col(b):
    return np.asarray(b, np.float32).reshape(-1, 1)


# ----------------------------------------------------------------------------
# Device program
# ----------------------------------------------------------------------------


def build_program(cfg, CQp, CQb, reps=1, skip=()):
    N, DA, DU, H, OUT = cfg["N"], cfg["DA"], cfg["DU"], cfg["H"], cfg["OUT"]
    n_cores, shard = cfg["n_cores"], cfg["shard"]
    NPAD = n_cores * shard
    QN = NPAD // 4
    NBLK = shard // 128
    G = _pick_G(NBLK)
    n_groups = (NBLK + G - 1) // G
    CBp, CBb = 4 * CQp, 4 * CQb
    DU1 = DU + 1  # ones row for bias
    DA1 = DA + 1
    KA = [(k, min(128, DA1 - k)) for k in range(0, DA1, 128)]
    TW = 512  # feature-major in-proj tile width
    n_tw = [(t, min(TW, shard - t)) for t in range(0, shard, TW)]

    nc = bacc.Bacc("TRN2", debug=False)

    # ---- I/O ----
    xaT = nc.dram_tensor("xaT", [DA1, shard], BF16, kind="ExternalInput")
    xuT = nc.dram_tensor("xuT", [DU1, shard], BF16, kind="ExternalInput")
    w_in_a = nc.dram_tensor("w_in_a", [DA1, H], BF16, kind="ExternalInput")
    w_in_u = nc.dram_tensor("w_in_u", [DU1, H], BF16, kind="ExternalInput")
    convw = {}
    for et in ("c1p", "c1b", "c2p"):
        convw[et] = (
            nc.dram_tensor(f"{et}_wlT", [H, H], BF16, kind="ExternalInput"),
            nc.dram_tensor(f"{et}_bl", [H, 1], FP32, kind="ExternalInput"),
            nc.dram_tensor(f"{et}_wrT", [H, H], BF16, kind="ExternalInput"),
        )
    # c2p wl staged in partitions 64:128 so its lhsT base matches aggs[64:128]
    c2p_wlT_hi = nc.dram_tensor("c2p_wlT_hi", [128, H], BF16, kind="ExternalInput")
    w_outT = nc.dram_tensor("w_outT", [H, OUT], BF16, kind="ExternalInput")
    b_out = nc.dram_tensor("b_out", [OUT, 1], FP32, kind="ExternalInput")
    iota_in = nc.dram_tensor("iota", [128, 128], BF16, kind="ExternalInput")
    ident_in = nc.dram_tensor("ident", [128, 128], BF16, kind="ExternalInput")
    NIDXp = n_groups * 4 * G * CQp * 8
    NIDXb = n_groups * 4 * G * CQb * 8
    idx_p = nc.dram_tensor("idx_p", [128, NIDXp], I16, kind="ExternalInput")
    slot_p = nc.dram_tensor("slot_p", [128, NBLK * CBp], FP32, kind="ExternalInput")
    rval_p = nc.dram_tensor("rval_p", [128, NBLK * CBp], FP32, kind="ExternalInput")
    idx_b = nc.dram_tensor("idx_b", [128, NIDXb], I16, kind="ExternalInput")
    slot_b = nc.dram_tensor("slot_b", [128, NBLK * CBb], FP32, kind="ExternalInput")
    rval_b = nc.dram_tensor("rval_b", [128, NBLK * CBb], FP32, kind="ExternalInput")
    out_d = nc.dram_tensor("out", [OUT, shard], FP32, kind="ExternalOutput")

    # internal HBM
    a_shard = nc.dram_tensor("a_shard", [shard, 128], BF16)
    uc_shard = nc.dram_tensor("uc_shard", [shard, 128], BF16)
    a_rm = nc.dram_tensor("a_rm", [NPAD, 128], BF16, addr_space="Shared")
    uc_rm = nc.dram_tensor("uc_rm", [NPAD, 128], BF16, addr_space="Shared")
    groups = [list(range(n_cores))]

    from contextlib import ExitStack

    with tile.TileContext(nc) as tc, ExitStack() as _stack:
        cpool = _stack.enter_context(tc.tile_pool(name="const", bufs=1))
        iota_sb = cpool.tile([128, 128], BF16, tag="iota")
        ident_sb = cpool.tile([128, 128], BF16, tag="ident")
        nc.sync.dma_start(iota_sb[:], iota_in[:])
        nc.sync.dma_start(ident_sb[:], ident_in[:])

        def load_const(t, shape, dtype, tag):
            s = cpool.tile(shape, dtype, tag=tag)
            nc.sync.dma_start(s[:], t[:])
            return s

        w_in_a_s = cpool.tile([128, len(KA), H], BF16, tag="w_in_a")
        for ki, (k0, kn) in enumerate(KA):
            nc.sync.dma_start(w_in_a_s[0:kn, ki, :], w_in_a[k0 : k0 + kn, :])
        w_in_u_s = load_const(w_in_u, [DU1, H], BF16, "w_in_u")
        convw_s = {}
        for et in ("c1p", "c1b", "c2p"):
            wlT, bl, wrT = convw[et]
            convw_s[et] = (
                load_const(wlT, [H, H], BF16, f"{et}_wlT"),
                load_const(bl, [H, 1], FP32, f"{et}_bl"),
                load_const(wrT, [H, H], BF16, f"{et}_wrT"),
            )
        c2p_wlT_hi_s = load_const(c2p_wlT_hi, [128, H], BF16, "c2p_wlT_hi")
        w_outT_s = load_const(w_outT, [H, OUT], BF16, "w_outT")
        b_out_s = load_const(b_out, [OUT, 1], FP32, "b_out")
        slot_p_s = load_const(slot_p, [128, NBLK * CBp], FP32, "slot_p")
        rval_p_s = load_const(rval_p, [128, NBLK * CBp], FP32, "rval_p")
        slot_b_s = load_const(slot_b, [128, NBLK * CBb], FP32, "slot_b")
        rval_b_s = load_const(rval_b, [128, NBLK * CBb], FP32, "rval_b")

        # resident feature-major node tables (own shard)
        uT_own = cpool.tile([H, shard], BF16, tag="uT_own")
        aT_own = cpool.tile([H, shard], BF16, tag="aT_own")

        # ------------------- stage 1: input projections -------------------
        def _inproj():
          with (
            tc.tile_pool(name="ip_ps", bufs=3, space="PSUM") as ip_ps,
            tc.tile_pool(name="nm_ps", bufs=3, space="PSUM") as nm_ps,
            tc.tile_pool(name="ip_sb", bufs=3) as ip_sb,
            tc.tile_pool(name="nm_sb", bufs=4) as nm_sb,
        ):
            # articles first: their node-major rows feed the a AllGather,
            # which gates pass 1.
            for t0, tw in n_tw:
                xt = ip_sb.tile([128, len(KA), TW], BF16, tag="xa")
                for ki, (k0, kn) in enumerate(KA):
                    nc.sync.dma_start(
                        xt[0:kn, ki, 0:tw], xaT[k0 : k0 + kn, t0 : t0 + tw]
                    )
                # feature-major: aT_own[:, tile] = relu(W' @ xa')
                ps = ip_ps.tile([H, TW], FP32, tag="ipps")
                for ki, (k0, kn) in enumerate(KA):
                    nc.tensor.matmul(
                        ps[:, 0:tw],
                        w_in_a_s[0:kn, ki, :],
                        xt[0:kn, ki, 0:tw],
                        start=(ki == 0),
                        stop=(ki == len(KA) - 1),
                    )
                nc.scalar.activation(aT_own[:, t0 : t0 + tw], ps[:, 0:tw], AF.Relu)
                # node-major: a_shard rows = relu(xa'^T @ W')
                for b0 in range(0, tw, 128):
                    ps2 = nm_ps.tile([128, H], FP32, tag="nmps")
                    for ki, (k0, kn) in enumerate(KA):
                        nc.tensor.matmul(
                            ps2[:],
                            xt[0:kn, ki, b0 : b0 + 128],
                            w_in_a_s[0:kn, ki, :],
                            start=(ki == 0),
                            stop=(ki == len(KA) - 1),
                        )
                    st = nm_sb.tile([128, H], BF16, tag="nmst")
                    nc.scalar.activation(st[:], ps2[:], AF.Relu)
                    nc.sync.dma_start(
                        a_shard[t0 + b0 : t0 + b0 + 128, 0:H], st[:]
                    )
            if "ag" not in skip:
                nc.gpsimd.collective_compute(
                    "AllGather", ALU.bypass, replica_groups=groups,
                    ins=[a_shard[:]], outs=[a_rm[:]],
                )
            for t0, tw in n_tw:
                xt = ip_sb.tile([DU1, TW], BF16, tag="xu")
                nc.sync.dma_start(xt[:, 0:tw], xuT[:, t0 : t0 + tw])
                ps = ip_ps.tile([H, TW], FP32, tag="ipps")
                nc.tensor.matmul(ps[:, 0:tw], w_in_u_s[:], xt[:, 0:tw])
                nc.scalar.activation(uT_own[:, t0 : t0 + tw], ps[:, 0:tw], AF.Relu)
                for b0 in range(0, tw, 128):
                    ps2 = nm_ps.tile([128, H], FP32, tag="nmps")
                    nc.tensor.matmul(ps2[:], xt[:, b0 : b0 + 128], w_in_u_s[:])
                    st = nm_sb.tile([128, H], BF16, tag="nmst")
                    nc.scalar.activation(st[:], ps2[:], AF.Relu)
                    nc.sync.dma_start(
                        uc_shard[t0 + b0 : t0 + b0 + 128, 0:H], st[:]
                    )

        # ------------------- conv passes -------------------
        def conv_pass(
            pools, gtable, idx_dram, slot_s, rval_s, CQ, pass2,
        ):
            """pass2=False: c1b (agg a over ei_pb -> u1 rows into uc_shard).
            pass2=True: c1p + c2p + head (agg [u|u1] over ei_posts)."""
            CB = 4 * CQ
            (msg_p, s_p, agg_ps, lin_ps, agg_sb, tp_ps, tp_sb,
             idx_pool, hd_ps, ost_p) = pools
            FW = 128 if pass2 else H
            for g in range(n_groups):
                g0 = g * G
                Gg = min(G, NBLK - g0)
                it = idx_pool.tile([128, 4 * G * CQ * 8], I16, tag="idxs")
                nc.sync.dma_start(
                    it[:, 0 : 4 * G * CQ * 8],
                    idx_dram[:, g * 4 * G * CQ * 8 : (g + 1) * 4 * G * CQ * 8],
                )
                msg = msg_p.tile([128, 4 * G * CQ, 128], BF16, tag="msg")
                if "gather" not in skip:
                    for q in range(4):
                        nc.gpsimd.dma_gather(
                            msg[:, q * G * CQ : q * G * CQ + Gg * CQ, :],
                            gtable[q * QN : (q + 1) * QN, :],
                            it[:, q * G * CQ * 8 : q * G * CQ * 8 + Gg * CQ * 8],
                            Gg * CQ * 128,
                            Gg * CQ * 128,
                            128,
                            # single_packet concatenation halves the per-
                            # descriptor HBM latency cost but hangs the SDMA
                            # for some larger per-engine packet shapes; G=1
                            # (CQ*128 idx per call) is validated safe.
                            single_packet=(
                                os.environ.get("KERNEL_SP", "auto") == "1"
                                or (os.environ.get("KERNEL_SP", "auto") == "auto"
                                    and G == 1)
                            ),
                        )
                else:
                    nc.vector.memset(msg[:], 0.0)
                if pass2:
                    ost = ost_p.tile([OUT, G * 128], FP32, tag="ost")
                for bl in range(Gg):
                    b = g0 + bl
                    agg = agg_ps.tile([FW, 128], FP32, tag="agg")
                    for c in range(CB):
                        q, cj = divmod(c, CQ)
                        S = s_p.tile([128, 128], BF16, tag="S")
                        nc.vector.tensor_scalar(
                            S[:],
                            iota_sb[:],
                            slot_s[:, b * CB + c : b * CB + c + 1],
                            rval_s[:, b * CB + c : b * CB + c + 1],
                            ALU.is_equal,
                            ALU.mult,
                        )
                        nc.tensor.matmul(
                            agg[:],
                            msg[:, q * G * CQ + bl * CQ + cj, 0:FW],
                            S[:],
                            start=(c == 0),
                            stop=(c == CB - 1),
                        )
                    aggs = agg_sb.tile([FW, 128], BF16, tag="aggs")
                    nc.scalar.copy(aggs[:], agg[:])
                    if not pass2:
                        wlT_s, bl_s, wrT_s = convw_s["c1b"]
                        lin = lin_ps.tile([H, 128], FP32, tag="lin")
                        nc.tensor.matmul(lin[:], wlT_s[:], aggs[:],
                                         start=True, stop=False)
                        nc.tensor.matmul(
                            lin[:], wrT_s[:], uT_own[:, b * 128 : (b + 1) * 128],
                            start=False, stop=True,
                        )
                        u1 = agg_sb.tile([H, 128], BF16, tag="u1")
                        nc.scalar.activation(u1[:], lin[:], AF.Relu, bias=bl_s[:])
                        tp = tp_ps.tile([128, H], BF16, tag="tpps")
                        nc.tensor.transpose(tp[:], u1[:], ident_sb[0:H, 0:H])
                        st = tp_sb.tile([128, H], BF16, tag="tpst")
                        nc.scalar.copy(st[:], tp[:])
                        nc.sync.dma_start(
                            uc_shard[b * 128 : (b + 1) * 128, H:128], st[:]
                        )
                    else:
                        wlT_s, bl_s, wrT_s = convw_s["c1p"]
                        lin = lin_ps.tile([H, 128], FP32, tag="lin")
                        nc.tensor.matmul(lin[:], wlT_s[:], aggs[0:H, :],
                                         start=True, stop=False)
                        nc.tensor.matmul(
                            lin[:], wrT_s[:], aT_own[:, b * 128 : (b + 1) * 128],
                            start=False, stop=True,
                        )
                        a1 = agg_sb.tile([H, 128], BF16, tag="a1")
                        nc.scalar.activation(a1[:], lin[:], AF.Relu, bias=bl_s[:])
                        _, bl2, wrT2 = convw_s["c2p"]
                        lin2 = lin_ps.tile([H, 128], FP32, tag="lin")
                        nc.tensor.matmul(lin2[:], c2p_wlT_hi_s[H:128, :],
                                         aggs[H:128, :], start=True, stop=False)
                        nc.tensor.matmul(lin2[:], wrT2[:], a1[:],
                                         start=False, stop=True)
                        a2 = agg_sb.tile([H, 128], BF16, tag="a2")
                        nc.vector.tensor_scalar_add(a2[:], lin2[:], bl2[:])
                        hp = hd_ps.tile([OUT, 128], FP32, tag="hdps")
                        nc.tensor.matmul(hp[:], w_outT_s[:], a2[:])
                        nc.vector.tensor_scalar_add(
                            ost[:, bl * 128 : (bl + 1) * 128], hp[:], b_out_s[:]
                        )
                if pass2:
                    nc.sync.dma_start(
                        out_d[:, g0 * 128 : g0 * 128 + Gg * 128],
                        ost[:, 0 : Gg * 128],
                    )

        def _convs():
          with (
            tc.tile_pool(name="msg", bufs=2) as msg_p,
            tc.tile_pool(name="S", bufs=6) as s_p,
            tc.tile_pool(name="agg_ps", bufs=2, space="PSUM") as agg_ps,
            tc.tile_pool(name="lin_ps", bufs=2, space="PSUM") as lin_ps,
            tc.tile_pool(name="agg_sb", bufs=3) as agg_sb,
            tc.tile_pool(name="tp_ps", bufs=2, space="PSUM") as tp_ps,
            tc.tile_pool(name="tp_sb", bufs=3) as tp_sb,
            tc.tile_pool(name="idxs", bufs=2) as idx_pool,
            tc.tile_pool(name="hd_ps", bufs=2, space="PSUM") as hd_ps,
            tc.tile_pool(name="ost", bufs=2) as ost_p,
        ):
            pools = (msg_p, s_p, agg_ps, lin_ps, agg_sb, tp_ps, tp_sb,
                     idx_pool, hd_ps, ost_p)
            # pass 1: c1b over ei_pb -> u1 rows into uc_shard[:, 64:128]
            conv_pass(pools, a_rm, idx_b, slot_b_s, rval_b_s, CQb, False)
            if "ag" not in skip:
                nc.gpsimd.collective_compute(
                    "AllGather", ALU.bypass, replica_groups=groups,
                    ins=[uc_shard[:]], outs=[uc_rm[:]],
                )
            # pass 2: c1p + c2p + head over ei_posts
            conv_pass(pools, uc_rm, idx_p, slot_p_s, rval_p_s, CQp, True)

        for _rep in range(reps):
            _inproj()
            if "convs" not in skip:
                _convs()

    nc.compile()
    return nc


# ----------------------------------------------------------------------------
# Entry point
# ----------------------------------------------------------------------------

_CACHE = {}


def build_in_maps(inputs, cfg, CQp, per_core_p, CQb, per_core_b):
    N, DA, DU, H = cfg["N"], cfg["DA"], cfg["DU"], cfg["H"]
    n_cores, shard = cfg["n_cores"], cfg["shard"]
    DA1, DU1 = DA + 1, DU + 1
    xa = np.asarray(inputs["x_article"], np.float32)
    xu = np.asarray(inputs["x_user"], np.float32)

    shared = dict(
        w_in_a=_lin_bias_bf16(inputs["w_in_a"], inputs["b_in_a"]),
        w_in_u=_lin_bias_bf16(inputs["w_in_u"], inputs["b_in_u"]),
        w_outT=_lin_bf16(inputs["w_out"]),
        b_out=_bias_col(inputs["b_out"]),
        iota=np.tile(np.arange(128, dtype=np.float32), (128, 1)).astype(BF16_NP),
        ident=np.eye(128, dtype=BF16_NP),
    )
    for et in ("c1p", "c1b", "c2p"):
        shared[f"{et}_wlT"] = _lin_bf16(inputs[f"{et}_wl"])
        shared[f"{et}_bl"] = _bias_col(inputs[f"{et}_bl"])
        shared[f"{et}_wrT"] = _lin_bf16(inputs[f"{et}_wr"])
    shared["c2p_wlT_hi"] = np.concatenate(
        [np.zeros((H, H), BF16_NP), _lin_bf16(inputs["c2p_wl"])], axis=0
    )

    in_maps = []
    for c in range(n_cores):
        c0, c1 = c * shard, min((c + 1) * shard, N)
        xaT_c = np.zeros((DA1, shard), BF16_NP)
        xaT_c[:DA, : c1 - c0] = xa[c0:c1].T.astype(BF16_NP)
        xaT_c[DA, :] = 1.0
        xuT_c = np.zeros((DU1, shard), BF16_NP)
        xuT_c[:DU, : c1 - c0] = xu[c0:c1].T.astype(BF16_NP)
        xuT_c[DU, :] = 1.0
        m = dict(shared)
        m["xaT"] = xaT_c
        m["xuT"] = xuT_c
        m["idx_p"] = per_core_p[c]["idx_w"]
        m["slot_p"] = per_core_p[c]["slot_w"]
        m["rval_p"] = per_core_p[c]["rval_w"]
        m["idx_b"] = per_core_b[c]["idx_w"]
        m["slot_b"] = per_core_b[c]["slot_w"]
        m["rval_b"] = per_core_b[c]["rval_w"]
        in_maps.append(m)
    return in_maps


def _run(inputs, cfg, trace=False, reps=1):
    N, n_cores, shard = cfg["N"], cfg["n_cores"], cfg["shard"]

    CQp, per_core_p = prep_edges(inputs["ei_posts"][0], inputs["ei_posts"][1], cfg)
    CQb, per_core_b = prep_edges(inputs["ei_pb"][0], inputs["ei_pb"][1], cfg)

    key = (tuple(sorted(cfg.items())), CQp, CQb, reps)
    if key not in _CACHE:
        _CACHE[key] = build_program(cfg, CQp, CQb, reps)
    nc = _CACHE[key]

    in_maps = build_in_maps(inputs, cfg, CQp, per_core_p, CQb, per_core_b)

    res = run_bass_kernel_spmd(nc, in_maps, list(range(n_cores)), trace=trace)
    outs = [res.results[c]["out"] for c in range(n_cores)]  # [2, shard] each
    full = np.concatenate(outs, axis=1)[:, :N].T.astype(np.float32)
    return np.ascontiguousarray(full), res


def kernel(**inputs):
    out, _ = _run(inputs, full_cfg(), trace=False)
    return out


# revision 26
# speedup vs baseline: 1.7625x; 1.3359x over previous
"""Trainium2 Bass kernel for hetero GNN (2x SAGEConv layers + in/out proj).

Full inputs in, full output out. Internally: dst-node sharding across 8
NeuronCores, edge bucketing by (dst block of 128, src quadrant) on host,
device-side gather via SWDGE dma_gather (batched over block groups),
segment-mean via one-hot matmul accumulated in PSUM, AllGather collectives
for the shared node tables.

v2 structure: the ei_posts edge list is gathered ONCE from a combined
[u | u1] node table (256B rows, fully used), so conv1-posts and conv2-posts
share gather descriptors, one-hot S tiles, and accumulation matmuls (the
c2p aggregation rides in partitions 64:128 of the same PSUM tile). Only two
AllGathers (a, u_comb) and two gather passes (ei_pb, ei_posts) remain.
Input projections emit both feature-major (for lin_r) and node-major (for
the gather tables) layouts directly via per-block matmuls with a ones-row
bias trick, eliminating on-chip transposes for u and a.
"""

import math

import numpy as np

import concourse.bacc as bacc
import concourse.bass as bass
import concourse.mybir as mybir
from concourse import tile
from concourse.bass_utils import run_bass_kernel_spmd

FP32 = mybir.dt.float32
BF16 = mybir.dt.bfloat16
I16 = mybir.dt.int16
AF = mybir.ActivationFunctionType
ALU = mybir.AluOpType

BF16_NP = mybir.dt.np(BF16)


def full_cfg():
    return dict(
        N=100000,
        E=1600000,
        DA=300,
        DU=64,
        H=64,
        OUT=2,
        n_cores=8,
        shard=12544,  # 98 * 128 per-core dst shard
        cq_min=5,
    )


import os


def _pick_G(nblk):
    """Blocks per dma_gather call. G=1 (one 128-dst block, CQ*128 indices per
    call) measures fastest on HW: it keeps single_packet concatenation safe,
    which halves the per-descriptor HBM latency cost; the extra SWDGE
    descriptor-generation calls hide entirely under the gather DMA time."""
    if os.environ.get("KERNEL_G"):
        return min(int(os.environ["KERNEL_G"]), nblk)
    return 1


# ----------------------------------------------------------------------------
# Host-side edge preprocessing
# ----------------------------------------------------------------------------


def prep_edges(src, dst, cfg):
    """Bucket edges by (dst block of 128, src quadrant); build gather index /
    one-hot slot / reciprocal-degree arrays per core.

    idx layout groups gather indices by (block-group, quadrant) so one
    dma_gather covers G blocks of one quadrant.

    Returns (CQ, per_core list of dicts with idx_w/slot_w/rval_w).
    """
    N, shard, n_cores = cfg["N"], cfg["shard"], cfg["n_cores"]
    NPAD = n_cores * shard
    QN = NPAD // 4
    assert QN < 32768, QN
    NBLK = shard // 128
    G = _pick_G(NBLK)
    n_groups = (NBLK + G - 1) // G

    src = np.asarray(src, dtype=np.int64)
    dst = np.asarray(dst, dtype=np.int64)
    deg = np.bincount(dst, minlength=N).astype(np.float64)
    recip = (1.0 / np.maximum(deg, 1.0)).astype(np.float32)

    blk = dst >> 7  # global 128-block id
    quad = src // QN
    cell = blk * 4 + quad
    n_cells = n_cores * NBLK * 4
    order = np.argsort(cell, kind="stable")
    c_src = src[order]
    c_dst = dst[order]
    c_cell = cell[order]
    starts = np.searchsorted(c_cell, np.arange(n_cells))
    cnts = np.bincount(c_cell, minlength=n_cells)
    CQ = max(cfg["cq_min"], int(math.ceil(cnts.max() / 128)))
    CB = 4 * CQ

    j = np.arange(len(c_src)) - starts[c_cell]  # position within cell
    loc_idx = (c_src - quad[order] * QN).astype(np.int16)
    slot_val = (c_dst & 127).astype(np.float32)
    rval_val = recip[c_dst]

    b_local_all = (c_cell // 4) % NBLK
    q_all = c_cell % 4
    core_all = c_cell // (4 * NBLK)

    per_core = []
    for c in range(n_cores):
        m = core_all == c
        bl = b_local_all[m]
        q = q_all[m]
        jj = j[m]
        g = bl // G
        bl_in_g = bl % G
        # gather idx array, 16-partition wrapped, replicated 8x;
        # grouped so (group, quadrant) segments are contiguous.
        idx_w = np.zeros((128, n_groups * 4 * G * CQ * 8), dtype=np.int16)
        col = ((g * 4 + q) * G + bl_in_g) * (CQ * 8) + jj // 16
        row = jj % 16
        for r in range(8):
            idx_w[row + 16 * r, col] = loc_idx[m]
        # slot / recip-val arrays: [128, NBLK*CB] bf16
        slot_w = np.full((128, NBLK * CB), 999.0, dtype=np.float32)
        rval_w = np.zeros((128, NBLK * CB), dtype=np.float32)
        colS = bl * CB + q * CQ + jj // 128
        rowS = jj % 128
        slot_w[rowS, colS] = slot_val[m]
        rval_w[rowS, colS] = rval_val[m]
        per_core.append(dict(idx_w=idx_w, slot_w=slot_w, rval_w=rval_w))
    return CQ, per_core


def _lin_bf16(w):
    """[out,in] fp32 -> lhsT layout [in,out] bf16."""
    return np.ascontiguousarray(np.asarray(w).T).astype(BF16_NP)


def _lin_bias_bf16(w, b):
    """[out,in] fp32 + [out] bias -> [in+1, out] bf16 with bias row."""
    w = np.asarray(w, np.float32)
    b = np.asarray(b, np.float32)
    return np.concatenate([w.T, b.reshape(1, -1)], axis=0).astype(BF16_NP)


def _bias_col(b):
    return np.asarray(b, np.float32).reshape(-1, 1)


# ----------------------------------------------------------------------------
# Device program
# ----------------------------------------------------------------------------


def build_program(cfg, CQp, CQb, reps=1, skip=()):
    N, DA, DU, H, OUT = cfg["N"], cfg["DA"], cfg["DU"], cfg["H"], cfg["OUT"]
    n_cores, shard = cfg["n_cores"], cfg["shard"]
    NPAD = n_cores * shard
    QN = NPAD // 4
    NBLK = shard // 128
    G = _pick_G(NBLK)
    n_groups = (NBLK + G - 1) // G
    CBp, CBb = 4 * CQp, 4 * CQb
    DU1 = DU + 1  # ones row for bias
    DA1 = DA + 1
    KA = [(k, min(128, DA1 - k)) for k in range(0, DA1, 128)]
    TW = 512  # feature-major in-proj tile width
    n_tw = [(t, min(TW, shard - t)) for t in range(0, shard, TW)]

    nc = bacc.Bacc("TRN2", debug=False)

    # ---- I/O ----
    xaT = nc.dram_tensor("xaT", [DA1, shard], BF16, kind="ExternalInput")
    xuT = nc.dram_tensor("xuT", [DU1, shard], BF16, kind="ExternalInput")
    w_in_a = nc.dram_tensor("w_in_a", [DA1, H], BF16, kind="ExternalInput")
    w_in_u = nc.dram_tensor("w_in_u", [DU1, H], BF16, kind="ExternalInput")
    convw = {}
    for et in ("c1p", "c1b", "c2p"):
        convw[et] = (
            nc.dram_tensor(f"{et}_wlT", [H, H], BF16, kind="ExternalInput"),
            nc.dram_tensor(f"{et}_bl", [H, 1], FP32, kind="ExternalInput"),
            nc.dram_tensor(f"{et}_wrT", [H, H], BF16, kind="ExternalInput"),
        )
    # c2p wl staged in partitions 64:128 so its lhsT base matches aggs[64:128]
    c2p_wlT_hi = nc.dram_tensor("c2p_wlT_hi", [128, H], BF16, kind="ExternalInput")
    w_outT = nc.dram_tensor("w_outT", [H, OUT], BF16, kind="ExternalInput")
    b_out = nc.dram_tensor("b_out", [OUT, 1], FP32, kind="ExternalInput")
    iota_in = nc.dram_tensor("iota", [128, 128], BF16, kind="ExternalInput")
    ident_in = nc.dram_tensor("ident", [128, 128], BF16, kind="ExternalInput")
    NIDXp = n_groups * 4 * G * CQp * 8
    NIDXb = n_groups * 4 * G * CQb * 8
    idx_p = nc.dram_tensor("idx_p", [128, NIDXp], I16, kind="ExternalInput")
    slot_p = nc.dram_tensor("slot_p", [128, NBLK * CBp], FP32, kind="ExternalInput")
    rval_p = nc.dram_tensor("rval_p", [128, NBLK * CBp], FP32, kind="ExternalInput")
    idx_b = nc.dram_tensor("idx_b", [128, NIDXb], I16, kind="ExternalInput")
    slot_b = nc.dram_tensor("slot_b", [128, NBLK * CBb], FP32, kind="ExternalInput")
    rval_b = nc.dram_tensor("rval_b", [128, NBLK * CBb], FP32, kind="ExternalInput")
    out_d = nc.dram_tensor("out", [OUT, shard], FP32, kind="ExternalOutput")

    # internal HBM
    a_shard = nc.dram_tensor("a_shard", [shard, 128], BF16)
    uc_shard = nc.dram_tensor("uc_shard", [shard, 128], BF16)
    a_rm = nc.dram_tensor("a_rm", [NPAD, 128], BF16, addr_space="Shared")
    uc_rm = nc.dram_tensor("uc_rm", [NPAD, 128], BF16, addr_space="Shared")
    groups = [list(range(n_cores))]

    from contextlib import ExitStack

    with tile.TileContext(nc) as tc, ExitStack() as _stack:
        cpool = _stack.enter_context(tc.tile_pool(name="const", bufs=1))
        iota_sb = cpool.tile([128, 128], BF16, tag="iota")
        ident_sb = cpool.tile([128, 128], BF16, tag="ident")
        nc.sync.dma_start(iota_sb[:], iota_in[:])
        nc.sync.dma_start(ident_sb[:], ident_in[:])

        def load_const(t, shape, dtype, tag):
            s = cpool.tile(shape, dtype, tag=tag)
            nc.sync.dma_start(s[:], t[:])
            return s

        w_in_a_s = cpool.tile([128, len(KA), H], BF16, tag="w_in_a")
        for ki, (k0, kn) in enumerate(KA):
            nc.sync.dma_start(w_in_a_s[0:kn, ki, :], w_in_a[k0 : k0 + kn, :])
        w_in_u_s = load_const(w_in_u, [DU1, H], BF16, "w_in_u")
        convw_s = {}
        for et in ("c1p", "c1b", "c2p"):
            wlT, bl, wrT = convw[et]
            convw_s[et] = (
                load_const(wlT, [H, H], BF16, f"{et}_wlT"),
                load_const(bl, [H, 1], FP32, f"{et}_bl"),
                load_const(wrT, [H, H], BF16, f"{et}_wrT"),
            )
        c2p_wlT_hi_s = load_const(c2p_wlT_hi, [128, H], BF16, "c2p_wlT_hi")
        w_outT_s = load_const(w_outT, [H, OUT], BF16, "w_outT")
        b_out_s = load_const(b_out, [OUT, 1], FP32, "b_out")
        slot_p_s = load_const(slot_p, [128, NBLK * CBp], FP32, "slot_p")
        rval_p_s = load_const(rval_p, [128, NBLK * CBp], FP32, "rval_p")
        slot_b_s = load_const(slot_b, [128, NBLK * CBb], FP32, "slot_b")
        rval_b_s = load_const(rval_b, [128, NBLK * CBb], FP32, "rval_b")

        # resident feature-major node tables (own shard)
        uT_own = cpool.tile([H, shard], BF16, tag="uT_own")
        aT_own = cpool.tile([H, shard], BF16, tag="aT_own")

        # ------------------- stage 1: input projections -------------------
        def _inproj():
          with (
            tc.tile_pool(name="ip_ps", bufs=3, space="PSUM") as ip_ps,
            tc.tile_pool(name="nm_ps", bufs=3, space="PSUM") as nm_ps,
            tc.tile_pool(name="ip_sb", bufs=3) as ip_sb,
            tc.tile_pool(name="nm_sb", bufs=4) as nm_sb,
        ):
            # articles first: their node-major rows feed the a AllGather,
            # which gates pass 1.
            for t0, tw in n_tw:
                xt = ip_sb.tile([128, len(KA), TW], BF16, tag="xa")
                for ki, (k0, kn) in enumerate(KA):
                    nc.sync.dma_start(
                        xt[0:kn, ki, 0:tw], xaT[k0 : k0 + kn, t0 : t0 + tw]
                    )
                # feature-major: aT_own[:, tile] = relu(W' @ xa')
                ps = ip_ps.tile([H, TW], FP32, tag="ipps")
                for ki, (k0, kn) in enumerate(KA):
                    nc.tensor.matmul(
                        ps[:, 0:tw],
                        w_in_a_s[0:kn, ki, :],
                        xt[0:kn, ki, 0:tw],
                        start=(ki == 0),
                        stop=(ki == len(KA) - 1),
                    )
                nc.scalar.activation(aT_own[:, t0 : t0 + tw], ps[:, 0:tw], AF.Relu)
                # node-major: a_shard rows = relu(xa'^T @ W')
                for b0 in range(0, tw, 128):
                    ps2 = nm_ps.tile([128, H], FP32, tag="nmps")
                    for ki, (k0, kn) in enumerate(KA):
                        nc.tensor.matmul(
                            ps2[:],
                            xt[0:kn, ki, b0 : b0 + 128],
                            w_in_a_s[0:kn, ki, :],
                            start=(ki == 0),
                            stop=(ki == len(KA) - 1),
                        )
                    st = nm_sb.tile([128, H], BF16, tag="nmst")
                    nc.scalar.activation(st[:], ps2[:], AF.Relu)
                    nc.sync.dma_start(
                        a_shard[t0 + b0 : t0 + b0 + 128, 0:H], st[:]
                    )
            if "ag" not in skip:
                nc.gpsimd.collective_compute(
                    "AllGather", ALU.bypass, replica_groups=groups,
                    ins=[a_shard[:]], outs=[a_rm[:]],
                )
            for t0, tw in n_tw:
                xt = ip_sb.tile([DU1, TW], BF16, tag="xu")
                nc.sync.dma_start(xt[:, 0:tw], xuT[:, t0 : t0 + tw])
                ps = ip_ps.tile([H, TW], FP32, tag="ipps")
                nc.tensor.matmul(ps[:, 0:tw], w_in_u_s[:], xt[:, 0:tw])
                nc.scalar.activation(uT_own[:, t0 : t0 + tw], ps[:, 0:tw], AF.Relu)
                for b0 in range(0, tw, 128):
                    ps2 = nm_ps.tile([128, H], FP32, tag="nmps")
                    nc.tensor.matmul(ps2[:], xt[:, b0 : b0 + 128], w_in_u_s[:])
                    st = nm_sb.tile([128, H], BF16, tag="nmst")
                    nc.scalar.activation(st[:], ps2[:], AF.Relu)
                    nc.sync.dma_start(
                        uc_shard[t0 + b0 : t0 + b0 + 128, 0:H], st[:]
                    )

        # ------------------- conv passes -------------------
        def conv_pass(
            pools, gtable, idx_dram, slot_s, rval_s, CQ, pass2,
        ):
            """pass2=False: c1b (agg a over ei_pb -> u1 rows into uc_shard).
            pass2=True: c1p + c2p + head (agg [u|u1] over ei_posts)."""
            CB = 4 * CQ
            (msg_p, s_p, agg_ps, lin_ps, agg_sb, tp_ps, tp_sb,
             idx_pool, hd_ps, ost_p) = pools
            FW = 128 if pass2 else H
            for g in range(n_groups):
                g0 = g * G
                Gg = min(G, NBLK - g0)
                it = idx_pool.tile([128, 4 * G * CQ * 8], I16, tag="idxs")
                nc.sync.dma_start(
                    it[:, 0 : 4 * G * CQ * 8],
                    idx_dram[:, g * 4 * G * CQ * 8 : (g + 1) * 4 * G * CQ * 8],
                )
                msg = msg_p.tile([128, 4 * G * CQ, 128], BF16, tag="msg")
                if "gather" not in skip:
                    for q in range(4):
                        nc.gpsimd.dma_gather(
                            msg[:, q * G * CQ : q * G * CQ + Gg * CQ, :],
                            gtable[q * QN : (q + 1) * QN, :],
                            it[:, q * G * CQ * 8 : q * G * CQ * 8 + Gg * CQ * 8],
                            Gg * CQ * 128,
                            Gg * CQ * 128,
                            128,
                            # single_packet concatenation halves the per-
                            # descriptor HBM latency cost but hangs the SDMA
                            # for some larger per-engine packet shapes; G=1
                            # (CQ*128 idx per call) is validated safe.
                            single_packet=(
                                os.environ.get("KERNEL_SP", "auto") == "1"
                                or (os.environ.get("KERNEL_SP", "auto") == "auto"
                                    and G == 1)
                            ),
                        )
                else:
                    nc.vector.memset(msg[:], 0.0)
                if pass2:
                    ost = ost_p.tile([OUT, G * 128], FP32, tag="ost")
                for bl in range(Gg):
                    b = g0 + bl
                    agg = agg_ps.tile([FW, 128], FP32, tag="agg")
                    for c in range(CB):
                        q, cj = divmod(c, CQ)
                        S = s_p.tile([128, 128], BF16, tag="S")
                        nc.vector.tensor_scalar(
                            S[:],
                            iota_sb[:],
                            slot_s[:, b * CB + c : b * CB + c + 1],
                            rval_s[:, b * CB + c : b * CB + c + 1],
                            ALU.is_equal,
                            ALU.mult,
                        )
                        nc.tensor.matmul(
                            agg[:],
                            msg[:, q * G * CQ + bl * CQ + cj, 0:FW],
                            S[:],
                            start=(c == 0),
                            stop=(c == CB - 1),
                        )
                    aggs = agg_sb.tile([FW, 128], BF16, tag="aggs")
                    nc.scalar.copy(aggs[:], agg[:])
                    if not pass2:
                        wlT_s, bl_s, wrT_s = convw_s["c1b"]
                        lin = lin_ps.tile([H, 128], FP32, tag="lin")
                        nc.tensor.matmul(lin[:], wlT_s[:], aggs[:],
                                         start=True, stop=False)
                        nc.tensor.matmul(
                            lin[:], wrT_s[:], uT_own[:, b * 128 : (b + 1) * 128],
                            start=False, stop=True,
                        )
                        u1 = agg_sb.tile([H, 128], BF16, tag="u1")
                        nc.scalar.activation(u1[:], lin[:], AF.Relu, bias=bl_s[:])
                        tp = tp_ps.tile([128, H], BF16, tag="tpps")
                        nc.tensor.transpose(tp[:], u1[:], ident_sb[0:H, 0:H])
                        st = tp_sb.tile([128, H], BF16, tag="tpst")
                        nc.scalar.copy(st[:], tp[:])
                        nc.sync.dma_start(
                            uc_shard[b * 128 : (b + 1) * 128, H:128], st[:]
                        )
                    else:
                        wlT_s, bl_s, wrT_s = convw_s["c1p"]
                        lin = lin_ps.tile([H, 128], FP32, tag="lin")
                        nc.tensor.matmul(lin[:], wlT_s[:], aggs[0:H, :],
                                         start=True, stop=False)
                        nc.tensor.matmul(
                            lin[:], wrT_s[:], aT_own[:, b * 128 : (b + 1) * 128],
                            start=False, stop=True,
                        )
                        a1 = agg_sb.tile([H, 128], BF16, tag="a1")
                        nc.scalar.activation(a1[:], lin[:], AF.Relu, bias=bl_s[:])
                        _, bl2, wrT2 = convw_s["c2p"]
                        lin2 = lin_ps.tile([H, 128], FP32, tag="lin")
                        nc.tensor.matmul(lin2[:], c2p_wlT_hi_s[H:128, :],
                                         aggs[H:128, :], start=True, stop=False)
                        nc.tensor.matmul(lin2[:], wrT2[:], a1[:],
                                         start=False, stop=True)
                        a2 = agg_sb.tile([H, 128], BF16, tag="a2")
                        nc.vector.tensor_scalar_add(a2[:], lin2[:], bl2[:])
                        hp = hd_ps.tile([OUT, 128], FP32, tag="hdps")
                        nc.tensor.matmul(hp[:], w_outT_s[:], a2[:])
                        nc.vector.tensor_scalar_add(
                            ost[:, bl * 128 : (bl + 1) * 128], hp[:], b_out_s[:]
                        )
                if pass2:
                    nc.sync.dma_start(
                        out_d[:, g0 * 128 : g0 * 128 + Gg * 128],
                        ost[:, 0 : Gg * 128],
                    )

        def _convs():
          with (
            tc.tile_pool(name="msg", bufs=2) as msg_p,
            tc.tile_pool(name="S", bufs=6) as s_p,
            tc.tile_pool(name="agg_ps", bufs=2, space="PSUM") as agg_ps,
            tc.tile_pool(name="lin_ps", bufs=2, space="PSUM") as lin_ps,
            tc.tile_pool(name="agg_sb", bufs=3) as agg_sb,
            tc.tile_pool(name="tp_ps", bufs=2, space="PSUM") as tp_ps,
            tc.tile_pool(name="tp_sb", bufs=3) as tp_sb,
            tc.tile_pool(name="idxs", bufs=2) as idx_pool,
            tc.tile_pool(name="hd_ps", bufs=2, space="PSUM") as hd_ps,
            tc.tile_pool(name="ost", bufs=2) as ost_p,
        ):
            pools = (msg_p, s_p, agg_ps, lin_ps, agg_sb, tp_ps, tp_sb,
                     idx_pool, hd_ps, ost_p)
            # pass 1: c1b over ei_pb -> u1 rows into uc_shard[:, 64:128]
            conv_pass(pools, a_rm, idx_b, slot_b_s, rval_b_s, CQb, False)
            if "ag" not in skip:
                nc.gpsimd.collective_compute(
                    "AllGather", ALU.bypass, replica_groups=groups,
                    ins=[uc_shard[:]], outs=[uc_rm[:]],
                )
            # pass 2: c1p + c2p + head over ei_posts
            conv_pass(pools, uc_rm, idx_p, slot_p_s, rval_p_s, CQp, True)

        for _rep in range(reps):
            _inproj()
            if "convs" not in skip:
                _convs()

    nc.compile()
    return nc


# ----------------------------------------------------------------------------
# Entry point
# ----------------------------------------------------------------------------

_CACHE = {}


def build_in_maps(inputs, cfg, CQp, per_core_p, CQb, per_core_b):
    N, DA, DU, H = cfg["N"], cfg["DA"], cfg["DU"], cfg["H"]
    n_cores, shard = cfg["n_cores"], cfg["shard"]
    DA1, DU1 = DA + 1, DU + 1
    xa = np.asarray(inputs["x_article"], np.float32)
    xu = np.asarray(inputs["x_user"], np.float32)

    shared = dict(
        w_in_a=_lin_bias_bf16(inputs["w_in_a"], inputs["b_in_a"]),
        w_in_u=_lin_bias_bf16(inputs["w_in_u"], inputs["b_in_u"]),
        w_outT=_lin_bf16(inputs["w_out"]),
        b_out=_bias_col(inputs["b_out"]),
        iota=np.tile(np.arange(128, dtype=np.float32), (128, 1)).astype(BF16_NP),
        ident=np.eye(128, dtype=BF16_NP),
    )
    for et in ("c1p", "c1b", "c2p"):
        shared[f"{et}_wlT"] = _lin_bf16(inputs[f"{et}_wl"])
        shared[f"{et}_bl"] = _bias_col(inputs[f"{et}_bl"])
        shared[f"{et}_wrT"] = _lin_bf16(inputs[f"{et}_wr"])
    shared["c2p_wlT_hi"] = np.concatenate(
        [np.zeros((H, H), BF16_NP), _lin_bf16(inputs["c2p_wl"])], axis=0
    )

    in_maps = []
    for c in range(n_cores):
        c0, c1 = c * shard, min((c + 1) * shard, N)
        xaT_c = np.zeros((DA1, shard), BF16_NP)
        xaT_c[:DA, : c1 - c0] = xa[c0:c1].T.astype(BF16_NP)
        xaT_c[DA, :] = 1.0
        xuT_c = np.zeros((DU1, shard), BF16_NP)
        xuT_c[:DU, : c1 - c0] = xu[c0:c1].T.astype(BF16_NP)
        xuT_c[DU, :] = 1.0
        m = dict(shared)
        m["xaT"] = xaT_c
        m["xuT"] = xuT_c
        m["idx_p"] = per_core_p[c]["idx_w"]
        m["slot_p"] = per_core_p[c]["slot_w"]
        m["rval_p"] = per_core_p[c]["rval_w"]
        m["idx_b"] = per_core_b[c]["idx_w"]
        m["slot_b"] = per_core_b[c]["slot_w"]
        m["rval_b"] = per_core_b[c]["rval_w"]
        in_maps.append(m)
    return in_maps


def _run(inputs, cfg, trace=False, reps=1):
    N, n_cores, shard = cfg["N"], cfg["n_cores"], cfg["shard"]

    CQp, per_core_p = prep_edges(inputs["ei_posts"][0], inputs["ei_posts"][1], cfg)
    CQb, per_core_b = prep_edges(inputs["ei_pb"][0], inputs["ei_pb"][1], cfg)

    key = (tuple(sorted(cfg.items())), CQp, CQb, reps)
    if key not in _CACHE:
        _CACHE[key] = build_program(cfg, CQp, CQb, reps)
    nc = _CACHE[key]

    in_maps = build_in_maps(inputs, cfg, CQp, per_core_p, CQb, per_core_b)

    res = run_bass_kernel_spmd(nc, in_maps, list(range(n_cores)), trace=trace)
    outs = [res.results[c]["out"] for c in range(n_cores)]  # [2, shard] each
    full = np.concatenate(outs, axis=1)[:, :N].T.astype(np.float32)
    return np.ascontiguousarray(full), res


def kernel(**inputs):
    out, _ = _run(inputs, full_cfg(), trace=False)
    return out
